# revision 41
# baseline (speedup 1.0000x reference)
"""Dilated (dil=2) 7x7 window self-attention, 4 heads x 32 dim, on 8 trn2 cores.

v2: spatial sharding over image rows (12 rows/core, 6-row halo), 4 cosets
(row/col parity) x 2 batches = 8 independent blocks per core.  Within a
block the coset grid is 6 query rows x 48 cols (NQ=288) attending over
12 key rows x 48 cols (NK=576) with a dense 7x7 window (|dr|,|dc| <= 3
in coset space; local key row kr attends query rows qr in [kr-6, kr]).

All matmuls bf16 (tolerance 2e-2 gives plenty of slack):
  - keys split into 6 column-chunks of 8 cols (96 keys = 12r x 8c each);
    queries touched by chunk j = 6 rows x 14 cols (global cols 8j-3..
    8j+10, clipped) -> logits unit [96 keys, 6x14=84] per (chunk, head).
  - phase 1: one matmul per (chunk, head): lhsT = K chunk [32, 96],
    rhs = Q window [32, 6, w] -> psum unit; 4 heads packed via
    tile_position rows.  24 units = 4 psum banks (6 units x 84 per bank).
  - exp: one ACT instruction per 2-bank half (12 units), no bias, no max
    subtraction (logits are tiny); writes bf16 attnT.
  - window mask: one bf16 multiply per half with a precomputed 0/1 mask
    (same for every unit).
  - key masking: V is projected from host-premultiplied x*m, so masked
    and padding keys contribute 0 to the numerator; the denominator is a
    matmul with lhsT = per-(block,chunk) key validity (eps for invalid)
    replicated x32, so invalid keys contribute ~eps.
  - phase 2: per chunk, 4 pO + 4 pS matmuls (col-tiled by head),
    accumulated across chunks into overlapping [32h, 6, w] psum windows.
  - normalize: reciprocal_approx_fast(pS) * pO -> bf16, then the 1x1
    output projection and a psum->sbuf fp32 copy + DMA out.

Blocks are software-pipelined: projections of block b+1 are emitted
between phase 1 and phase 2 of block b so the PE never waits on the
ACT/DVE exp/mask chain.
"""

import numpy as np

HEADS, D, WIN, DIL = 4, 32, 7, 2
B, C, H, W = 2, 128, 96, 96
CORES = 8
CR, KR, W2 = 6, 12, 48            # coset query rows / key rows (halo) / cols
NQ, NK = CR * W2, KR * W2         # 288, 576
NBLK = B * 4                      # (batch, coset) blocks per core
NCH = 6                           # key column chunks of 8
SCALE = float(1.0 / np.sqrt(D))
EPS = 1e-5                        # denominator weight for invalid keys
_PIPE = True                      # software-pipeline blocks
_NRUN = NBLK                      # blocks to emit in no-pipe debug mode
_STAGES = 5                       # no-pipe debug: how many stages to emit

_prog = None


def _chunk_geo(j):
    """(gl0, w, l0): global q-col start, width, offset in 14-col frame."""
    gl0 = max(0, 8 * j - 3)
    gl1 = min(W2 - 1, 8 * j + 10)
    return gl0, gl1 - gl0 + 1, gl0 - (8 * j - 3)


def _unit_off(j, h):
    """attnT / psum offsets of unit (chunk j, head h).  Bank h holds head
    h's six 84-wide units — concurrent head-tiles must drain to DISTINCT
    psum banks (same-bank same-partition concurrent drains fault the HW)."""
    att = h * 504 + j * 84
    pl = h * 512 + j * 84
    return att, pl


def _build_program():
    import concourse.bass as bass
    import concourse.tile as tile
    from concourse import mybir

    nc = bass.Bass("TRN2", target_bir_lowering=False, debug=False,
                   num_devices=CORES)
    f32 = mybir.dt.float32
    bf16 = mybir.dt.bfloat16

    xq_d = nc.dram_tensor("xq", [128, NBLK * NQ], bf16, kind="ExternalInput").ap()
    xk_d = nc.dram_tensor("xk", [128, NBLK * NK], bf16, kind="ExternalInput").ap()
    xm_d = nc.dram_tensor("xm", [128, NBLK * NK], bf16, kind="ExternalInput").ap()
    wm_d = nc.dram_tensor("wm", [128, 1008], bf16, kind="ExternalInput").ap()
    mk_d = nc.dram_tensor("mk", [128, NBLK * NCH * 32], bf16,
                          kind="ExternalInput").ap()
    wq_d = nc.dram_tensor("wq", [128, 128], bf16, kind="ExternalInput").ap()
    wk_d = nc.dram_tensor("wk", [128, 128], bf16, kind="ExternalInput").ap()
    wv_d = nc.dram_tensor("wv", [128, 128], bf16, kind="ExternalInput").ap()
    wp_d = nc.dram_tensor("wp", [128, 128], bf16, kind="ExternalInput").ap()
    out_d = nc.dram_tensor("out", [128, NBLK * NQ], f32,
                           kind="ExternalOutput").ap()

    with tile.TileContext(nc) as tc:
        with tc.tile_pool(name="cst", bufs=1) as cst, \
             tc.tile_pool(name="att", bufs=2) as attp, \
             tc.tile_pool(name="nrm", bufs=2) as nrm, \
             tc.tile_pool(name="psL", bufs=1, space="PSUM") as psL, \
             tc.tile_pool(name="psO", bufs=1, space="PSUM") as psO, \
             tc.tile_pool(name="psP", bufs=2, space="PSUM") as psP:

            # DMA issue cost is ~600ns per dma_start on an engine queue —
            # spread issuance across idle engines, earliest-needed first.
            w_q = cst.tile([128, 128], bf16)
            nc.scalar.dma_start(out=w_q[:], in_=wq_d[:])
            w_k = cst.tile([128, 128], bf16)
            nc.scalar.dma_start(out=w_k[:], in_=wk_d[:])
            w_v = cst.tile([128, 128], bf16)
            nc.scalar.dma_start(out=w_v[:], in_=wv_d[:])
            w_p = cst.tile([128, 128], bf16)
            nc.scalar.dma_start(out=w_p[:], in_=wp_d[:])
            WMt = cst.tile([128, 1008], bf16)
            nc.gpsimd.dma_start(out=WMt[:], in_=wm_d[:])
            MKt = cst.tile([128, NBLK * NCH * 32], bf16)
            nc.gpsimd.dma_start(out=MKt[:], in_=mk_d[:])

            Xq = cst.tile([128, NBLK * NQ], bf16)
            Xk = cst.tile([128, NBLK * NK], bf16)
            Xm = cst.tile([128, NBLK * NK], bf16)
            for q in range(2):
                sl = slice(q * NBLK * NK // 2, (q + 1) * NBLK * NK // 2)
                nc.sync.dma_start(out=Xk[:, sl], in_=xk_d[:, sl])
                nc.gpsimd.dma_start(out=Xm[:, sl], in_=xm_d[:, sl])
                s2 = slice(q * NBLK * NQ // 2, (q + 1) * NBLK * NQ // 2)
                nc.sync.dma_start(out=Xq[:, s2], in_=xq_d[:, s2])

            # zero the psL ring slots once so exp of never-written lanes
            # stays bounded
            plz0 = psL.tile([128, 1024], f32, tag="plA")
            nc.vector.memset(plz0[:], 0.0)
            plz1 = psL.tile([128, 1024], f32, tag="plB")
            nc.vector.memset(plz1[:], 0.0)

            # per-block state carried between pipeline stages
            st = [dict() for _ in range(NBLK)]

            Qall = cst.tile([128, NBLK * NQ], bf16)
            Kall = cst.tile([128, NBLK * NK], bf16)
            VTall = cst.tile([128, NBLK * NCH * 128], bf16)

            def proj_all(sched, rot):
                """Projection steps (512-wide matmuls), copies alternating
                between ACT and DVE.  rot=True borrows idle attention psum
                banks (startup only)."""
                eng = [nc.scalar, nc.vector] if rot else [nc.vector, nc.scalar]
                ncopy = 0
                # During the projection phase the attention psum banks are
                # idle: rotate over 6 slots so matmuls never wait on copies.
                slots = ([(psP, "pp"), (psP, "pp"), (psL, "plA"),
                          (psL, "plB"), (psO, "po"), (psO, "ps")]
                         if rot else [(psP, "pp")])
                nalloc = [0]

                def ptile(name):
                    pool, tag = slots[nalloc[0] % len(slots)]
                    nalloc[0] += 1
                    return pool.tile([128, 512], f32, tag=tag, name=name)

                def emit(pt, dst_ap, n, par=128):
                    nonlocal ncopy
                    e = eng[ncopy % 2]
                    if e is nc.scalar:
                        e.copy(out=dst_ap, in_=pt[0:par, :n])
                    else:
                        e.tensor_copy(dst_ap, pt[0:par, :n])
                    ncopy += 1

                def kstep(i):
                    n = min(512, NBLK * NK - i * 512)
                    pk = ptile(f"pk{i}")
                    nc.tensor.matmul(out=pk[:, :n], lhsT=w_k[:],
                                     rhs=Xk[:, i * 512:i * 512 + n],
                                     start=True, stop=True)
                    emit(pk, Kall[:, i * 512:i * 512 + n], n)

                def qstep(i):
                    n = min(512, NBLK * NQ - i * 512)
                    pq = ptile(f"pq{i}")
                    nc.tensor.matmul(out=pq[:, :n], lhsT=w_q[:],
                                     rhs=Xq[:, i * 512:i * 512 + n],
                                     start=True, stop=True)
                    emit(pq, Qall[:, i * 512:i * 512 + n], n)

                def vstep(g):
                    pv = ptile(f"pv{g}")
                    for c in range(4):
                        u = g * 4 + c
                        nc.tensor.matmul(out=pv[0:96, c * 128:(c + 1) * 128],
                                         lhsT=Xm[:, u * 96:(u + 1) * 96],
                                         rhs=w_v[:], start=True, stop=True)
                    emit(pv, VTall[0:96, g * 512:(g + 1) * 512], 512, par=96)

                for kind, i in sched:
                    {"k": kstep, "q": qstep, "v": vstep}[kind](i)

            def ph1(b, half):
                """Logits for heads {2*half, 2*half+1}: each head drains to
                its own psum bank (concurrent same-bank same-partition
                drains fault the HW)."""
                s = st[b]
                pl = psL.tile([128, 1024], mybir.dt.float32,
                              tag="plA" if half == 0 else "plB",
                              name=f"pl{b}_{half}")
                s[f"pl{half}"] = pl
                for j in range(NCH):
                    gl0, w, l0 = _chunk_geo(j)
                    for hh in range(2):
                        h = 2 * half + hh
                        dst = pl[0:96, hh * 512 + j * 84:
                                 hh * 512 + j * 84 + 84] \
                            .rearrange("p (r c) -> p r c", c=14)[:, :, l0:l0 + w]
                        lhsT = Kall[32 * h:32 * h + 32,
                                    b * NK + j * 96:b * NK + (j + 1) * 96]
                        rhs = Qall[32 * h:32 * h + 32,
                                   b * NQ:(b + 1) * NQ] \
                            .rearrange("p (r c) -> p r c", c=W2)[:, :, gl0:gl0 + w]
                        nc.tensor.matmul(out=dst, lhsT=lhsT, rhs=rhs,
                                         start=True, stop=True,
                                         tile_position=(32 * h, 0))

            def expmask(b, half):
                s = st[b]
                if half == 0:
                    s["att"] = attp.tile([128, 2016], mybir.dt.bfloat16,
                                         tag="att", name=f"att{b}")
                att = s["att"]
                src = s[f"pl{half}"][0:96, :].rearrange("p (k x) -> p k x",
                                                        k=2)[:, :, 0:504]
                dst = att[0:96, half * 1008:(half + 1) * 1008] \
                    .rearrange("p (k x) -> p k x", k=2)
                nc.scalar.activation(out=dst, in_=src,
                                     func=mybir.ActivationFunctionType.Exp,
                                     scale=SCALE)
                if _STAGES >= 2:
                    sl = slice(half * 1008, (half + 1) * 1008)
                    nc.vector.tensor_mul(out=att[0:96, sl],
                                         in0=att[0:96, sl],
                                         in1=WMt[0:96, 0:1008])

            def ph2(b, jlist):
                s = st[b]
                if "pO" not in s:
                    s["pO"] = psO.tile([128, 512], mybir.dt.float32, tag="po",
                                       name=f"pO{b}")
                    s["pS"] = psO.tile([128, 512], mybir.dt.float32, tag="ps",
                                       name=f"pS{b}")
                pO, pS = s["pO"], s["pS"]
                for j in jlist:
                    gl0, w, l0 = _chunk_geo(j)
                    for h in range(4):
                        attoff, _ = _unit_off(j, h)
                        rhs = s["att"][0:96, attoff:attoff + 84] \
                            .rearrange("p (r c) -> p r c", c=14)[:, :, l0:l0 + w]
                        dstO = pO[32 * h:32 * h + 32, :NQ] \
                            .rearrange("p (r c) -> p r c", c=W2)[:, :, gl0:gl0 + w]
                        nc.tensor.matmul(
                            out=dstO,
                            lhsT=VTall[0:96, b * 768 + j * 128 + 32 * h:
                                       b * 768 + j * 128 + 32 * h + 32],
                            rhs=rhs, start=(j == 0), stop=(j == 5),
                            tile_position=(0, 32 * h))
                        dstS = pS[32 * h:32 * h + 32, :NQ] \
                            .rearrange("p (r c) -> p r c", c=W2)[:, :, gl0:gl0 + w]
                        nc.tensor.matmul(
                            out=dstS,
                            lhsT=MKt[0:96, (b * NCH + j) * 32:
                                     (b * NCH + j) * 32 + 32],
                            rhs=rhs, start=(j == 0), stop=(j == 5),
                            tile_position=(0, 32 * h))

            def norm(b):
                # 1/pS as exp(-ln pS) on ACT (Ln/Exp share one act table);
                # DVE reciprocal is ~1.9us, this is ~0.85us off-DVE.
                s = st[b]
                lnS = nrm.tile([128, NQ], mybir.dt.float32, tag="lns",
                               name=f"lnS{b}")
                nc.scalar.activation(out=lnS[:], in_=s["pS"][:, :NQ],
                                     func=mybir.ActivationFunctionType.Ln)
                rcpS = nrm.tile([128, NQ], mybir.dt.float32, tag="rcp",
                                name=f"rcpS{b}")
                nc.scalar.activation(out=rcpS[:], in_=lnS[:],
                                     func=mybir.ActivationFunctionType.Exp,
                                     scale=-1.0)
                onrm = nrm.tile([128, NQ], mybir.dt.bfloat16, tag="on",
                                name=f"on{b}")
                nc.vector.tensor_mul(out=onrm[:], in0=s["pO"][:, :NQ],
                                     in1=rcpS[:])
                s["on"] = onrm

            def final(b):
                s = st[b]
                pf = psP.tile([128, 512], mybir.dt.float32, tag="pp",
                              name=f"pf{b}")
                nc.tensor.matmul(out=pf[:, :NQ], lhsT=w_p[:], rhs=s["on"][:],
                                 start=True, stop=True)
                osb = nrm.tile([128, NQ], mybir.dt.float32, tag="osb",
                               name=f"osb{b}")
                nc.vector.tensor_copy(osb[:], pf[:, :NQ])
                nc.sync.dma_start(out=out_d[:, b * NQ:(b + 1) * NQ],
                                  in_=osb[:])
                st[b] = {}

            UPFRONT = [("k", 0), ("k", 1), ("q", 0), ("v", 0), ("v", 1),
                       ("k", 2), ("q", 1), ("v", 2)]
            LOOPSTEPS = [[("k", 3), ("q", 2), ("v", 3)],
                         [("k", 4), ("v", 4), ("v", 5)],
                         [("k", 5), ("v", 6), ("v", 7)],
                         [("k", 6), ("q", 3), ("v", 8)],
                         [("k", 7), ("v", 9), ("v", 10)],
                         [("k", 8), ("q", 4), ("v", 11)]]
            proj_all(UPFRONT, rot=True)
            if not _PIPE:
                for stp in LOOPSTEPS:
                    proj_all(stp, rot=False)
            if not _PIPE:
                for b in range(_NRUN):
                    if _STAGES >= 1.3:
                        ph1(b, 0)
                        ph1(b, 1)
                    if _STAGES >= 1.6:
                        expmask(b, 0)
                        expmask(b, 1)
                    if _STAGES >= 3:
                        ph2(b, [0, 1, 2])
                        ph2(b, [3, 4, 5])
                    if _STAGES >= 4:
                        norm(b)
                    if _STAGES >= 5:
                        final(b)
            else:
                # 3-stage software pipeline per iteration `it`:
                #   ph1/exp/mask(it) | ph2/norm(it-1) | final(it-2)
                for it in range(NBLK + 2):
                    if it < NBLK:
                        ph1(it, 0)
                        expmask(it, 0)
                        ph1(it, 1)
                        expmask(it, 1)
                    if it < len(LOOPSTEPS):
                        proj_all(LOOPSTEPS[it], rot=False)
                    if 0 <= it - 1 < NBLK:
                        ph2(it - 1, [0, 1, 2])
                        ph2(it - 1, [3, 4, 5])
                        norm(it - 1)
                    if 0 <= it - 2 < NBLK:
                        final(it - 2)

    _split_multi_waits(nc)
    return nc


def _split_multi_waits(nc):
    """This walrus build rejects >1 sem wait per instruction: move extra
    waits onto dedicated single-wait NoOps inserted just before."""
    import copy
    from concourse import mybir

    tmpl = nc.sync.nop(nofuse=True, hint="wsplit_template").ins
    bb0 = nc.cur_bb.bb
    bb0.instructions = [i for i in bb0.instructions if i.name != tmpl.name]
    tmpl = copy.deepcopy(tmpl)

    ctr = 0
    for f in nc.m.functions:
        for bb in f.blocks:
            insts = list(bb.instructions)
            new, changed = [], False
            for inst in insts:
                si = getattr(inst, "sync_info", None)
                waits = list(si.on_wait) if si is not None and si.on_wait else []
                if len(waits) > 1:
                    for w in waits[:-1]:
                        ctr += 1
                        nop = copy.deepcopy(tmpl)
                        nop.name = f"I-wsplit{ctr}"
                        nop.engine = inst.engine
                        nop.sync_info = mybir.SyncInfo(on_wait=[w], on_update=[])
                        new.append(nop)
                    si.on_wait = [waits[-1]]
                    changed = True
                new.append(inst)
            if changed:
                bb.instructions = new


def _host_prep(x, m):
    """Per-core inputs: xq [128, NBLK*NQ] row-major center rows; xk/xm
    [128, NBLK*NK] chunk-major (key p = (j, kr, kc')); mk [128, NBLK*6*32]."""
    import ml_dtypes
    bf = ml_dtypes.bfloat16
    # chunk-major permutation of a 576-key block
    perm = np.array([kr * W2 + 8 * j + kc
                     for j in range(NCH) for kr in range(KR)
                     for kc in range(8)], np.int64)
    xqs, xks, xms, mks = [], [], [], []
    mf = (m > 0).astype(np.float32)
    for k in range(CORES):
        r0 = 12 * k - 6
        xpad = np.zeros((B, C, 24, W), np.float32)
        mpad = np.zeros((B, 1, 24, W), np.float32)
        lo, hi = max(0, r0), min(H, r0 + 24)
        xpad[:, :, lo - r0:hi - r0] = x[:, :, lo:hi]
        mpad[:, :, lo - r0:hi - r0] = mf[:, :, lo:hi]
        xmp = xpad * mpad

        def coset(t, ch):
            v = t.reshape(B, ch, KR, 2, W2, 2).transpose(1, 0, 3, 5, 2, 4)
            return v.reshape(ch, NBLK, NK)

        xc = coset(xpad, C)                       # [C, NBLK, NK] row-major
        xq = xc[:, :, 144:144 + NQ].reshape(C, NBLK * NQ)
        xk = xc[:, :, perm].reshape(C, NBLK * NK)
        xm = coset(xmp, C)[:, :, perm].reshape(C, NBLK * NK)
        xqs.append(np.ascontiguousarray(xq).astype(bf))
        xks.append(np.ascontiguousarray(xk).astype(bf))
        xms.append(np.ascontiguousarray(xm).astype(bf))
        mc = coset(mpad, 1)[0][:, perm].reshape(NBLK, NCH, 96)
        mk = np.zeros((128, NBLK * NCH * 32), np.float32)
        vals = np.where(mc > 0, 1.0, EPS)         # [NBLK, NCH, 96]
        mk[0:96] = np.repeat(vals.reshape(NBLK * NCH, 96).T, 32, axis=1)
        mks.append(mk.astype(bf))
    return xqs, xks, xms, mks


def _host_wm():
    """[128, 1008] bf16: 0/1 window mask, unit layout [12 units][6 qr][14 lc],
    key partition p = kr*8 + kc'."""
    import ml_dtypes
    kr = np.arange(KR)[:, None, None, None]
    kc = np.arange(8)[None, :, None, None]
    qr = np.arange(CR)[None, None, :, None]
    lc = np.arange(14)[None, None, None, :]
    win = ((kr - qr >= 0) & (kr - qr <= 6) & (lc >= kc) & (lc <= kc + 6))
    unit = win.reshape(96, 84).astype(np.float32)
    wm = np.zeros((128, 1008), np.float32)
    wm[0:96] = np.tile(unit, (1, 12))
    return wm.astype(ml_dtypes.bfloat16)


def _make_in_maps(x, m, Wq, Wk, Wv, Wp):
    import ml_dtypes
    bf = ml_dtypes.bfloat16
    xqs, xks, xms, mks = _host_prep(np.asarray(x, np.float32),
                                    np.asarray(m, np.int32))
    base = {
        "wm": _host_wm(),
        "wq": np.ascontiguousarray(np.asarray(Wq, np.float32).T).astype(bf),
        "wk": np.ascontiguousarray(np.asarray(Wk, np.float32).T).astype(bf),
        "wv": np.ascontiguousarray(np.asarray(Wv, np.float32).T).astype(bf),
        "wp": np.ascontiguousarray(np.asarray(Wp, np.float32).T).astype(bf),
    }
    return [{**base, "xq": xqs[k], "xk": xks[k], "xm": xms[k], "mk": mks[k]}
            for k in range(CORES)]


def kernel(x, m, Wq, Wk, Wv, Wp):
    global _prog
    from concourse.bass_utils import run_bass_kernel_spmd

    if _prog is None:
        _prog = _build_program()

    in_maps = _make_in_maps(x, m, Wq, Wk, Wv, Wp)
    res = run_bass_kernel_spmd(_prog, in_maps, list(range(CORES)))

    full = np.zeros((B, C, H, W), np.float32)
    for k in range(CORES):
        oc = res.results[k]["out"].reshape(C, B, 2, 2, CR, W2)
        o = oc.transpose(1, 0, 4, 2, 5, 3).reshape(B, C, 12, 96)
        full[:, :, 12 * k:12 * k + 12, :] = o
    return full


# revision 43
# speedup vs baseline: 1.0306x; 1.0306x over previous
"""Dilated (dil=2) 7x7 window self-attention, 4 heads x 32 dim, on 8 trn2 cores.

v2: spatial sharding over image rows (12 rows/core, 6-row halo), 4 cosets
(row/col parity) x 2 batches = 8 independent blocks per core.  Within a
block the coset grid is 6 query rows x 48 cols (NQ=288) attending over
12 key rows x 48 cols (NK=576) with a dense 7x7 window (|dr|,|dc| <= 3
in coset space; local key row kr attends query rows qr in [kr-6, kr]).

All matmuls bf16 (tolerance 2e-2 gives plenty of slack):
  - keys split into 6 column-chunks of 8 cols (96 keys = 12r x 8c each);
    queries touched by chunk j = 6 rows x 14 cols (global cols 8j-3..
    8j+10, clipped) -> logits unit [96 keys, 6x14=84] per (chunk, head).
  - phase 1: one matmul per (chunk, head): lhsT = K chunk [32, 96],
    rhs = Q window [32, 6, w] -> psum unit; 4 heads packed via
    tile_position rows.  24 units = 4 psum banks (6 units x 84 per bank).
  - exp: one ACT instruction per 2-bank half (12 units), no bias, no max
    subtraction (logits are tiny); writes bf16 attnT.
  - window mask: one bf16 multiply per half with a precomputed 0/1 mask
    (same for every unit).
  - key masking: V is projected from host-premultiplied x*m, so masked
    and padding keys contribute 0 to the numerator; the denominator is a
    matmul with lhsT = per-(block,chunk) key validity (eps for invalid)
    replicated x32, so invalid keys contribute ~eps.
  - phase 2: per chunk, 4 pO + 4 pS matmuls (col-tiled by head),
    accumulated across chunks into overlapping [32h, 6, w] psum windows.
  - normalize: reciprocal_approx_fast(pS) * pO -> bf16, then the 1x1
    output projection and a psum->sbuf fp32 copy + DMA out.

Blocks are software-pipelined: projections of block b+1 are emitted
between phase 1 and phase 2 of block b so the PE never waits on the
ACT/DVE exp/mask chain.
"""

import numpy as np

HEADS, D, WIN, DIL = 4, 32, 7, 2
B, C, H, W = 2, 128, 96, 96
CORES = 8
CR, KR, W2 = 6, 12, 48            # coset query rows / key rows (halo) / cols
NQ, NK = CR * W2, KR * W2         # 288, 576
NBLK = B * 4                      # (batch, coset) blocks per core
NCH = 6                           # key column chunks of 8
SCALE = float(1.0 / np.sqrt(D))
EPS = 1e-5                        # denominator weight for invalid keys
_PIPE = True                      # software-pipeline blocks
_NRUN = NBLK                      # blocks to emit in no-pipe debug mode
_STAGES = 5                       # no-pipe debug: how many stages to emit

_prog = None


def _chunk_geo(j):
    """(gl0, w, l0): global q-col start, width, offset in 14-col frame."""
    gl0 = max(0, 8 * j - 3)
    gl1 = min(W2 - 1, 8 * j + 10)
    return gl0, gl1 - gl0 + 1, gl0 - (8 * j - 3)


def _unit_off(j, h):
    """attnT / psum offsets of unit (chunk j, head h).  Bank h holds head
    h's six 84-wide units — concurrent head-tiles must drain to DISTINCT
    psum banks (same-bank same-partition concurrent drains fault the HW)."""
    att = h * 504 + j * 84
    pl = h * 512 + j * 84
    return att, pl


def _build_program():
    import concourse.bass as bass
    import concourse.tile as tile
    from concourse import mybir

    nc = bass.Bass("TRN2", target_bir_lowering=False, debug=False,
                   num_devices=CORES)
    f32 = mybir.dt.float32
    bf16 = mybir.dt.bfloat16

    xq_d = nc.dram_tensor("xq", [128, NBLK * NQ], bf16, kind="ExternalInput").ap()
    xk_d = nc.dram_tensor("xk", [128, NBLK * NK], bf16, kind="ExternalInput").ap()
    xm_d = nc.dram_tensor("xm", [128, NBLK * NK], bf16, kind="ExternalInput").ap()
    wm_d = nc.dram_tensor("wm", [128, 1008], bf16, kind="ExternalInput").ap()
    mk_d = nc.dram_tensor("mk", [128, NBLK * NCH * 32], bf16,
                          kind="ExternalInput").ap()
    wq_d = nc.dram_tensor("wq", [128, 128], bf16, kind="ExternalInput").ap()
    wk_d = nc.dram_tensor("wk", [128, 128], bf16, kind="ExternalInput").ap()
    wv_d = nc.dram_tensor("wv", [128, 128], bf16, kind="ExternalInput").ap()
    wp_d = nc.dram_tensor("wp", [128, 128], bf16, kind="ExternalInput").ap()
    out_d = nc.dram_tensor("out", [128, NBLK * NQ], f32,
                           kind="ExternalOutput").ap()

    with tile.TileContext(nc) as tc:
        with tc.tile_pool(name="cst", bufs=1) as cst, \
             tc.tile_pool(name="att", bufs=2) as attp, \
             tc.tile_pool(name="nrm", bufs=2) as nrm, \
             tc.tile_pool(name="psL", bufs=1, space="PSUM") as psL, \
             tc.tile_pool(name="psO", bufs=1, space="PSUM") as psO, \
             tc.tile_pool(name="psP", bufs=2, space="PSUM") as psP:

            # DMA issue cost is ~600ns per dma_start on an engine queue —
            # spread issuance across idle engines, earliest-needed first.
            w_q = cst.tile([128, 128], bf16)
            nc.sync.dma_start(out=w_q[:], in_=wq_d[:])
            w_k = cst.tile([128, 128], bf16)
            nc.sync.dma_start(out=w_k[:], in_=wk_d[:])
            w_v = cst.tile([128, 128], bf16)
            nc.gpsimd.dma_start(out=w_v[:], in_=wv_d[:])
            w_p = cst.tile([128, 128], bf16)
            nc.gpsimd.dma_start(out=w_p[:], in_=wp_d[:])
            WMt = cst.tile([128, 1008], bf16)
            nc.gpsimd.dma_start(out=WMt[:], in_=wm_d[:])
            MKt = cst.tile([128, NBLK * NCH * 32], bf16)
            nc.gpsimd.dma_start(out=MKt[:], in_=mk_d[:])

            Xq = cst.tile([128, NBLK * NQ], bf16)
            Xk = cst.tile([128, NBLK * NK], bf16)
            Xm = cst.tile([128, NBLK * NK], bf16)
            for q in range(2):
                sl = slice(q * NBLK * NK // 2, (q + 1) * NBLK * NK // 2)
                nc.sync.dma_start(out=Xk[:, sl], in_=xk_d[:, sl])
                nc.gpsimd.dma_start(out=Xm[:, sl], in_=xm_d[:, sl])
                s2 = slice(q * NBLK * NQ // 2, (q + 1) * NBLK * NQ // 2)
                nc.sync.dma_start(out=Xq[:, s2], in_=xq_d[:, s2])

            # zero the psL ring slots once so exp of never-written lanes
            # stays bounded
            plz0 = psL.tile([128, 1024], f32, tag="plA")
            nc.vector.memset(plz0[:], 0.0)
            plz1 = psL.tile([128, 1024], f32, tag="plB")
            nc.vector.memset(plz1[:], 0.0)

            # per-block state carried between pipeline stages
            st = [dict() for _ in range(NBLK)]

            Qall = cst.tile([128, NBLK * NQ], bf16)
            Kall = cst.tile([128, NBLK * NK], bf16)
            VTall = cst.tile([128, NBLK * NCH * 128], bf16)

            def proj_all():
                """All projections up front: 512-wide matmuls, copies
                alternating between ACT and DVE."""
                eng = [nc.scalar, nc.vector]
                ncopy = 0
                # During the projection phase the attention psum banks are
                # idle: rotate over 6 slots so matmuls never wait on copies.
                slots = [(psP, "pp"), (psP, "pp"), (psL, "plA"),
                         (psL, "plB"), (psO, "po"), (psO, "ps")]
                nalloc = [0]

                def ptile(name):
                    pool, tag = slots[nalloc[0] % 6]
                    nalloc[0] += 1
                    return pool.tile([128, 512], f32, tag=tag, name=name)

                def emit(pt, dst_ap, n, par=128):
                    nonlocal ncopy
                    e = eng[ncopy % 2]
                    if e is nc.scalar:
                        e.copy(out=dst_ap, in_=pt[0:par, :n])
                    else:
                        e.tensor_copy(dst_ap, pt[0:par, :n])
                    ncopy += 1

                def kstep(i):
                    n = min(512, NBLK * NK - i * 512)
                    pk = ptile(f"pk{i}")
                    nc.tensor.matmul(out=pk[:, :n], lhsT=w_k[:],
                                     rhs=Xk[:, i * 512:i * 512 + n],
                                     start=True, stop=True)
                    emit(pk, Kall[:, i * 512:i * 512 + n], n)

                def qstep(i):
                    n = min(512, NBLK * NQ - i * 512)
                    pq = ptile(f"pq{i}")
                    nc.tensor.matmul(out=pq[:, :n], lhsT=w_q[:],
                                     rhs=Xq[:, i * 512:i * 512 + n],
                                     start=True, stop=True)
                    emit(pq, Qall[:, i * 512:i * 512 + n], n)

                def vstep(g):
                    pv = ptile(f"pv{g}")
                    for c in range(4):
                        u = g * 4 + c
                        nc.tensor.matmul(out=pv[0:96, c * 128:(c + 1) * 128],
                                         lhsT=Xm[:, u * 96:(u + 1) * 96],
                                         rhs=w_v[:], start=True, stop=True)
                    emit(pv, VTall[0:96, g * 512:(g + 1) * 512], 512, par=96)

                # block-0-first interleave so the attention loop starts early
                sched = ([("k", 0), ("k", 1), ("q", 0), ("v", 0), ("v", 1),
                          ("q", 1)] +
                         [x for i in range(2, 9)
                          for x in [("k", i), ("v", i), ("v", i + 5)]
                          ] + [("q", i) for i in range(2, 5)])
                seen = set()
                for kind, i in sched:
                    if (kind, i) in seen or (kind == "v" and i > 11):
                        continue
                    seen.add((kind, i))
                    {"k": kstep, "q": qstep, "v": vstep}[kind](i)
                for i in range(12):
                    if ("v", i) not in seen:
                        vstep(i)

            def ph1(b, half):
                """Logits for heads {2*half, 2*half+1}: each head drains to
                its own psum bank (concurrent same-bank same-partition
                drains fault the HW)."""
                s = st[b]
                pl = psL.tile([128, 1024], mybir.dt.float32,
                              tag="plA" if half == 0 else "plB",
                              name=f"pl{b}_{half}")
                s[f"pl{half}"] = pl
                for j in range(NCH):
                    gl0, w, l0 = _chunk_geo(j)
                    for hh in range(2):
                        h = 2 * half + hh
                        dst = pl[0:96, hh * 512 + j * 84:
                                 hh * 512 + j * 84 + 84] \
                            .rearrange("p (r c) -> p r c", c=14)[:, :, l0:l0 + w]
                        lhsT = Kall[32 * h:32 * h + 32,
                                    b * NK + j * 96:b * NK + (j + 1) * 96]
                        rhs = Qall[32 * h:32 * h + 32,
                                   b * NQ:(b + 1) * NQ] \
                            .rearrange("p (r c) -> p r c", c=W2)[:, :, gl0:gl0 + w]
                        nc.tensor.matmul(out=dst, lhsT=lhsT, rhs=rhs,
                                         start=True, stop=True,
                                         tile_position=(32 * h, 0))

            def expmask(b, half):
                s = st[b]
                if half == 0:
                    s["att"] = attp.tile([128, 2016], mybir.dt.bfloat16,
                                         tag="att", name=f"att{b}")
                att = s["att"]
                src = s[f"pl{half}"][0:96, :].rearrange("p (k x) -> p k x",
                                                        k=2)[:, :, 0:504]
                dst = att[0:96, half * 1008:(half + 1) * 1008] \
                    .rearrange("p (k x) -> p k x", k=2)
                nc.scalar.activation(out=dst, in_=src,
                                     func=mybir.ActivationFunctionType.Exp,
                                     scale=SCALE)
                if _STAGES >= 2:
                    sl = slice(half * 1008, (half + 1) * 1008)
                    nc.vector.tensor_mul(out=att[0:96, sl],
                                         in0=att[0:96, sl],
                                         in1=WMt[0:96, 0:1008])

            def ph2(b, jlist):
                s = st[b]
                if "pO" not in s:
                    s["pO"] = psO.tile([128, 512], mybir.dt.float32, tag="po",
                                       name=f"pO{b}")
                    s["pS"] = psO.tile([128, 512], mybir.dt.float32, tag="ps",
                                       name=f"pS{b}")
                pO, pS = s["pO"], s["pS"]
                for j in jlist:
                    gl0, w, l0 = _chunk_geo(j)
                    for h in range(4):
                        attoff, _ = _unit_off(j, h)
                        rhs = s["att"][0:96, attoff:attoff + 84] \
                            .rearrange("p (r c) -> p r c", c=14)[:, :, l0:l0 + w]
                        dstO = pO[32 * h:32 * h + 32, :NQ] \
                            .rearrange("p (r c) -> p r c", c=W2)[:, :, gl0:gl0 + w]
                        nc.tensor.matmul(
                            out=dstO,
                            lhsT=VTall[0:96, b * 768 + j * 128 + 32 * h:
                                       b * 768 + j * 128 + 32 * h + 32],
                            rhs=rhs, start=(j == 0), stop=(j == 5),
                            tile_position=(0, 32 * h))
                        dstS = pS[32 * h:32 * h + 32, :NQ] \
                            .rearrange("p (r c) -> p r c", c=W2)[:, :, gl0:gl0 + w]
                        nc.tensor.matmul(
                            out=dstS,
                            lhsT=MKt[0:96, (b * NCH + j) * 32:
                                     (b * NCH + j) * 32 + 32],
                            rhs=rhs, start=(j == 0), stop=(j == 5),
                            tile_position=(0, 32 * h))

            def norm(b):
                # 1/pS as exp(-ln pS) on ACT (Ln/Exp share one act table);
                # DVE reciprocal is ~1.9us, this is ~0.85us off-DVE.
                s = st[b]
                lnS = nrm.tile([128, NQ], mybir.dt.float32, tag="lns",
                               name=f"lnS{b}")
                nc.scalar.activation(out=lnS[:], in_=s["pS"][:, :NQ],
                                     func=mybir.ActivationFunctionType.Ln)
                rcpS = nrm.tile([128, NQ], mybir.dt.float32, tag="rcp",
                                name=f"rcpS{b}")
                nc.scalar.activation(out=rcpS[:], in_=lnS[:],
                                     func=mybir.ActivationFunctionType.Exp,
                                     scale=-1.0)
                onrm = nrm.tile([128, NQ], mybir.dt.bfloat16, tag="on",
                                name=f"on{b}")
                nc.vector.tensor_mul(out=onrm[:], in0=s["pO"][:, :NQ],
                                     in1=rcpS[:])
                s["on"] = onrm

            def final(b):
                s = st[b]
                pf = psP.tile([128, 512], mybir.dt.float32, tag="pp",
                              name=f"pf{b}")
                nc.tensor.matmul(out=pf[:, :NQ], lhsT=w_p[:], rhs=s["on"][:],
                                 start=True, stop=True)
                osb = nrm.tile([128, NQ], mybir.dt.float32, tag="osb",
                               name=f"osb{b}")
                nc.vector.tensor_copy(osb[:], pf[:, :NQ])
                nc.sync.dma_start(out=out_d[:, b * NQ:(b + 1) * NQ],
                                  in_=osb[:])
                st[b] = {}

            proj_all()
            if not _PIPE:
                for b in range(_NRUN):
                    if _STAGES >= 1.3:
                        ph1(b, 0)
                        ph1(b, 1)
                    if _STAGES >= 1.6:
                        expmask(b, 0)
                        expmask(b, 1)
                    if _STAGES >= 3:
                        ph2(b, [0, 1, 2])
                        ph2(b, [3, 4, 5])
                    if _STAGES >= 4:
                        norm(b)
                    if _STAGES >= 5:
                        final(b)
            else:
                # 3-stage software pipeline per iteration `it`:
                #   ph1/exp/mask(it) | ph2/norm(it-1) | final(it-2)
                for it in range(NBLK + 2):
                    if it < NBLK:
                        ph1(it, 0)
                        expmask(it, 0)
                        ph1(it, 1)
                        expmask(it, 1)
                    if 0 <= it - 1 < NBLK:
                        ph2(it - 1, [0, 1, 2])
                        ph2(it - 1, [3, 4, 5])
                        norm(it - 1)
                        if it - 1 >= NBLK - 2:
                            final(it - 1)
                    if 0 <= it - 2 < NBLK - 2:
                        final(it - 2)

    _split_multi_waits(nc)
    return nc


def _split_multi_waits(nc):
    """This walrus build rejects >1 sem wait per instruction: move extra
    waits onto dedicated single-wait NoOps inserted just before."""
    import copy
    from concourse import mybir

    tmpl = nc.sync.nop(nofuse=True, hint="wsplit_template").ins
    bb0 = nc.cur_bb.bb
    bb0.instructions = [i for i in bb0.instructions if i.name != tmpl.name]
    tmpl = copy.deepcopy(tmpl)

    ctr = 0
    for f in nc.m.functions:
        for bb in f.blocks:
            insts = list(bb.instructions)
            new, changed = [], False
            for inst in insts:
                si = getattr(inst, "sync_info", None)
                waits = list(si.on_wait) if si is not None and si.on_wait else []
                if len(waits) > 1:
                    for w in waits[:-1]:
                        ctr += 1
                        nop = copy.deepcopy(tmpl)
                        nop.name = f"I-wsplit{ctr}"
                        nop.engine = inst.engine
                        nop.sync_info = mybir.SyncInfo(on_wait=[w], on_update=[])
                        new.append(nop)
                    si.on_wait = [waits[-1]]
                    changed = True
                new.append(inst)
            if changed:
                bb.instructions = new


def _host_prep(x, m):
    """Per-core inputs: xq [128, NBLK*NQ] row-major center rows; xk/xm
    [128, NBLK*NK] chunk-major (key p = (j, kr, kc')); mk [128, NBLK*6*32]."""
    import ml_dtypes
    bf = ml_dtypes.bfloat16
    # chunk-major permutation of a 576-key block
    perm = np.array([kr * W2 + 8 * j + kc
                     for j in range(NCH) for kr in range(KR)
                     for kc in range(8)], np.int64)
    xqs, xks, xms, mks = [], [], [], []
    mf = (m > 0).astype(np.float32)
    for k in range(CORES):
        r0 = 12 * k - 6
        xpad = np.zeros((B, C, 24, W), np.float32)
        mpad = np.zeros((B, 1, 24, W), np.float32)
        lo, hi = max(0, r0), min(H, r0 + 24)
        xpad[:, :, lo - r0:hi - r0] = x[:, :, lo:hi]
        mpad[:, :, lo - r0:hi - r0] = mf[:, :, lo:hi]
        xmp = xpad * mpad

        def coset(t, ch):
            v = t.reshape(B, ch, KR, 2, W2, 2).transpose(1, 0, 3, 5, 2, 4)
            return v.reshape(ch, NBLK, NK)

        xc = coset(xpad, C)                       # [C, NBLK, NK] row-major
        xq = xc[:, :, 144:144 + NQ].reshape(C, NBLK * NQ)
        xk = xc[:, :, perm].reshape(C, NBLK * NK)
        xm = coset(xmp, C)[:, :, perm].reshape(C, NBLK * NK)
        xqs.append(np.ascontiguousarray(xq).astype(bf))
        xks.append(np.ascontiguousarray(xk).astype(bf))
        xms.append(np.ascontiguousarray(xm).astype(bf))
        mc = coset(mpad, 1)[0][:, perm].reshape(NBLK, NCH, 96)
        mk = np.zeros((128, NBLK * NCH * 32), np.float32)
        vals = np.where(mc > 0, 1.0, EPS)         # [NBLK, NCH, 96]
        mk[0:96] = np.repeat(vals.reshape(NBLK * NCH, 96).T, 32, axis=1)
        mks.append(mk.astype(bf))
    return xqs, xks, xms, mks


def _host_wm():
    """[128, 1008] bf16: 0/1 window mask, unit layout [12 units][6 qr][14 lc],
    key partition p = kr*8 + kc'."""
    import ml_dtypes
    kr = np.arange(KR)[:, None, None, None]
    kc = np.arange(8)[None, :, None, None]
    qr = np.arange(CR)[None, None, :, None]
    lc = np.arange(14)[None, None, None, :]
    win = ((kr - qr >= 0) & (kr - qr <= 6) & (lc >= kc) & (lc <= kc + 6))
    unit = win.reshape(96, 84).astype(np.float32)
    wm = np.zeros((128, 1008), np.float32)
    wm[0:96] = np.tile(unit, (1, 12))
    return wm.astype(ml_dtypes.bfloat16)


def _make_in_maps(x, m, Wq, Wk, Wv, Wp):
    import ml_dtypes
    bf = ml_dtypes.bfloat16
    xqs, xks, xms, mks = _host_prep(np.asarray(x, np.float32),
                                    np.asarray(m, np.int32))
    base = {
        "wm": _host_wm(),
        "wq": np.ascontiguousarray(np.asarray(Wq, np.float32).T).astype(bf),
        "wk": np.ascontiguousarray(np.asarray(Wk, np.float32).T).astype(bf),
        "wv": np.ascontiguousarray(np.asarray(Wv, np.float32).T).astype(bf),
        "wp": np.ascontiguousarray(np.asarray(Wp, np.float32).T).astype(bf),
    }
    return [{**base, "xq": xqs[k], "xk": xks[k], "xm": xms[k], "mk": mks[k]}
            for k in range(CORES)]


def kernel(x, m, Wq, Wk, Wv, Wp):
    global _prog
    from concourse.bass_utils import run_bass_kernel_spmd

    if _prog is None:
        _prog = _build_program()

    in_maps = _make_in_maps(x, m, Wq, Wk, Wv, Wp)
    res = run_bass_kernel_spmd(_prog, in_maps, list(range(CORES)))

    full = np.zeros((B, C, H, W), np.float32)
    for k in range(CORES):
        oc = res.results[k]["out"].reshape(C, B, 2, 2, CR, W2)
        o = oc.transpose(1, 0, 4, 2, 5, 3).reshape(B, C, 12, 96)
        full[:, :, 12 * k:12 * k + 12, :] = o
    return full
